# revision 12
# baseline (speedup 1.0000x reference)
"""Trainium2 Bass kernel: CrossAttention3D (B=4, Lq=Lk=4096, D=256) on 8 NeuronCores.

Sharding: core c handles batch c//2, decoder-query half c%2 (2048 queries),
with the full encoder sequence for that batch and replicated projections.

v2: fp8(e4m3) + DoubleRow matmuls everywhere; transposed layouts prepared on
host so no PE transposes are needed; output is computed as O^T [d, q] and
transposed back on host.

Per-core dataflow (all d-major tensors are [128, 2, seq] with d = h*128 + p):
  QT = Wq^T-chunks (stationary, DR) @ xdT (moving)   -> [d, 2048] fp8 (+bq)
  KT = Wk^T-chunks @ xeT                             -> [d, 4096] fp8 (+bk)
  V' = xeT-chunks (stationary) @ Wvo (moving)        -> [k, 256]+ones fp8
  per (qc=512 queries, ktpair=256 keys):
    st[128k,2,512q] = KT-slice (stationary) @ QT     (two DR matmuls)
    pt = exp(st/16 - 2) fp8                          (one ACT instr per pair)
    O^T[128d, 512q] += V'pair-chunk (stationary) @ pt  (two DR matmuls)
    denom[1, 512q]  += V'pair-ones  (stationary) @ pt  (one DR matmul)
  epilogue: outT = O^T * (1/denom broadcast) + (xdT + bo2)  (DVE + gpsimd)
Host: folds Wv@Wo, bv@Wo+bo; pre-transposes/casts inputs; transposes outT back.
"""
import os
import sys

import numpy as np

for _p in ("/opt/trn_rl_repo", os.path.expanduser("~/.axon_site/_ro/trn_rl_repo")):
    if os.path.isdir(_p) and _p not in sys.path:
        sys.path.insert(0, _p)

B, LQ, LK, D = 4, 4096, 4096, 256
NCORES = 8
QCHUNK = LQ // 2          # queries per core
P = 128
SCALE = 1.0 / 16.0        # 1/sqrt(D)
ESHIFT = -2.0             # exp shift (cancels in softmax division)
NQC = QCHUNK // 512       # 4 query chunks of 512 per core
NKT = LK // P             # 32 key tiles
NPAIR = NKT // 2          # 16 key-tile pairs (DoubleRow contracts 256 keys)
VW = 272                  # V' row stride (256 data + 1 ones + pad, %16==0)

_STATE = {}
VARIANT = os.environ.get("KV", "full")


def _build(repeat=1):
    from contextlib import ExitStack

    import concourse.tile as tile
    from concourse import bacc, mybir

    f32 = mybir.dt.float32
    f8 = mybir.dt.float8e4
    AF = mybir.ActivationFunctionType
    ALU = mybir.AluOpType
    DR = mybir.MatmulPerfMode.DoubleRow

    nc = bacc.Bacc(trn_type="TRN2")
    xdT8_d = nc.dram_tensor("xdT8", [P, 2, QCHUNK], f8, kind="ExternalInput")
    xdT32_d = nc.dram_tensor("xdT32", [P, 2, QCHUNK], f32, kind="ExternalInput")
    xeT8_d = nc.dram_tensor("xeT8", [P, 2, LK], f8, kind="ExternalInput")
    wq_d = nc.dram_tensor("wq8", [P, 2, D], f8, kind="ExternalInput")
    wk_d = nc.dram_tensor("wk8", [P, 2, D], f8, kind="ExternalInput")
    wvo_d = nc.dram_tensor("wvo8", [P, 2, D], f8, kind="ExternalInput")
    bq_d = nc.dram_tensor("bq2", [P, 2], f32, kind="ExternalInput")
    bk_d = nc.dram_tensor("bk2", [P, 2], f32, kind="ExternalInput")
    outT_d = nc.dram_tensor("outT", [2, P, QCHUNK], f32, kind="ExternalOutput")

    with tile.TileContext(nc) as tc:
        loop_ctx = ExitStack()
        if repeat > 1:
            loop_ctx.enter_context(tc.For_i(0, repeat, 1))
        with (
            tc.tile_pool(name="singles", bufs=1) as singles,
            tc.tile_pool(name="ptp", bufs=4) as ptp,
            tc.tile_pool(name="outp", bufs=2) as outp,
            tc.tile_pool(name="recp", bufs=2) as recp,
        ):
            # ---- persistent SBUF tensors, loaded straight from DRAM ----
            xdT8 = singles.tile([P, 2, QCHUNK], f8)
            nc.sync.dma_start(out=xdT8, in_=xdT8_d[:])
            xeT8 = singles.tile([P, 2, LK], f8)
            nc.sync.dma_start(out=xeT8, in_=xeT8_d[:])
            xdT32 = singles.tile([P, 2, QCHUNK], f32)
            nc.sync.dma_start(out=xdT32, in_=xdT32_d[:])
            wq8 = singles.tile([P, 2, D], f8)
            nc.sync.dma_start(out=wq8, in_=wq_d[:])
            wk8 = singles.tile([P, 2, D], f8)
            nc.sync.dma_start(out=wk8, in_=wk_d[:])
            wvo8 = singles.tile([P, 2, D], f8)
            nc.sync.dma_start(out=wvo8, in_=wvo_d[:])
            bq2 = singles.tile([P, 2], f32)
            nc.sync.dma_start(out=bq2, in_=bq_d[:])
            bk2 = singles.tile([P, 2], f32)
            nc.sync.dma_start(out=bk2, in_=bk_d[:])

            expb = singles.tile([P, 1], f32)
            nc.vector.memset(expb, ESHIFT)

            QT8 = singles.tile([P, 2, QCHUNK], f8)
            KT8 = singles.tile([P, 2, LK], f8)
            ones8 = singles.tile([P, 2, P], f8)
            nc.vector.memset(ones8, 1.0)
            # V' [k_lo, pair, parity, d] with ones in column 256
            Vp8 = singles.tile([P, NPAIR, 2, VW], f8)
            nc.vector.memset(Vp8[:, :, :, D:D + 1], 1.0)

            # ---------------- projections ----------------
            with tc.tile_pool(name="pj_ps", bufs=3, space="PSUM") as pj_ps:
                for qc in range(NQC):
                    for c in range(2):
                        pj = pj_ps.tile([P, 512], f32, tag="pj")
                        nc.tensor.matmul(pj, wq8[:, :, c * P:(c + 1) * P],
                                         xdT8[:, :, qc * 512:(qc + 1) * 512],
                                         start=True, stop=True, perf_mode=DR)
                        nc.vector.tensor_scalar_add(
                            QT8[:, c, qc * 512:(qc + 1) * 512], pj, bq2[:, c:c + 1])
                for kc in range(LK // 512):
                    for c in range(2):
                        pj = pj_ps.tile([P, 512], f32, tag="pj")
                        nc.tensor.matmul(pj, wk8[:, :, c * P:(c + 1) * P],
                                         xeT8[:, :, kc * 512:(kc + 1) * 512],
                                         start=True, stop=True, perf_mode=DR)
                        nc.vector.tensor_scalar_add(
                            KT8[:, c, kc * 512:(kc + 1) * 512], pj, bk2[:, c:c + 1])
                for kt in range(NKT):
                    pv = pj_ps.tile([P, 512], f32, tag="pj")
                    nc.tensor.matmul(pv[:, :D], xeT8[:, :, kt * P:(kt + 1) * P],
                                     wvo8, start=True, stop=True, perf_mode=DR)
                    nc.vector.tensor_copy(Vp8[:, kt // 2, kt % 2, 0:D], pv[:, :D])

            # ---------------- attention main loop ----------------
            with (
                tc.tile_pool(name="st_ps", bufs=2, space="PSUM") as st_ps,
                tc.tile_pool(name="o_ps", bufs=1, space="PSUM") as o_ps_pool,
                tc.tile_pool(name="dn_ps", bufs=2, space="PSUM") as dn_ps_pool,
            ):
                o_ps = o_ps_pool.tile([P, 2, 512], f32)
                for qc in range(NQC):
                    qsl = slice(qc * 512, (qc + 1) * 512)
                    dn_ps = dn_ps_pool.tile([P, 512], f32, tag="dn")

                    def issue_o(p, pt):
                        for c in range(2):
                            nc.tensor.matmul(o_ps[:, c, :],
                                             Vp8[:, p, :, c * P:(c + 1) * P], pt,
                                             start=(p == 0), stop=(p == NPAIR - 1),
                                             perf_mode=DR)
                        nc.tensor.matmul(dn_ps, ones8, pt,
                                         start=(p == 0), stop=(p == NPAIR - 1),
                                         perf_mode=DR)

                    def issue_o_v(p, pt):
                        if VARIANT == "no_o":
                            nc.tensor.matmul(dn_ps, ones8, pt,
                                             start=(p == 0), stop=(p == NPAIR - 1),
                                             perf_mode=DR)
                        elif VARIANT == "no_dn":
                            for c in range(2):
                                nc.tensor.matmul(o_ps[:, c, :],
                                                 Vp8[:, p, :, c * P:(c + 1) * P], pt,
                                                 start=(p == 0),
                                                 stop=(p == NPAIR - 1),
                                                 perf_mode=DR)
                        elif VARIANT == "s_only":
                            pass
                        else:
                            issue_o(p, pt)

                    pe_pt = None
                    if VARIANT == "pe_only":
                        pe_pt = ptp.tile([P, 2, 512], f8, tag="pt")
                        nc.vector.memset(pe_pt[:, 0:1, 0:1], 0.125)
                    pts = [None] * NPAIR
                    for p in range(NPAIR):
                        st = st_ps.tile([P, 2, 512], f32, tag="st")
                        for j in range(2):
                            kt = 2 * p + j
                            nc.tensor.matmul(st[:, j, :],
                                             KT8[:, :, kt * P:(kt + 1) * P],
                                             QT8[:, :, qsl],
                                             start=True, stop=True, perf_mode=DR)
                        if VARIANT == "pe_only":
                            pts[p] = pe_pt
                        else:
                            pt = ptp.tile([P, 2, 512], f8, tag="pt")
                            nc.scalar.activation(pt, st, AF.Exp,
                                                 bias=expb, scale=SCALE)
                            pts[p] = pt
                        # software pipeline: PE consumes the PREVIOUS pair's
                        # exp output so it never waits on this pair's ACT
                        if p > 0:
                            issue_o_v(p - 1, pts[p - 1])
                    issue_o_v(NPAIR - 1, pts[NPAIR - 1])
                    # drain O^T out of PSUM (frees banks for next qc asap)
                    o32 = outp.tile([P, 2, 512], f32, tag="o32")
                    if VARIANT in ("s_only", "no_o"):
                        nc.vector.memset(o32, 1.0)
                    else:
                        nc.vector.tensor_copy(o32, o_ps)
                    # epilogue: divide by denom, add residual(+bo2), store
                    recB = recp.tile([P, 512], f32, tag="recB")
                    if VARIANT in ("no_dn", "s_only"):
                        nc.vector.memset(recB, 1.0)
                    else:
                        nc.vector.reciprocal(recB, dn_ps)
                    for c in range(2):
                        outf = outp.tile([P, 512], f32, tag="outf")
                        nc.vector.tensor_mul(outf, o32[:, c, :], recB)
                        nc.vector.tensor_add(outf, outf, xdT32[:, c, qsl])
                        nc.sync.dma_start(out=outT_d[c, :, qsl], in_=outf)

        loop_ctx.close()

    nc.finalize()
    return nc


def _get_nc(repeat=1):
    key = f"nc{repeat}"
    if key not in _STATE:
        _STATE[key] = _build(repeat)
    return _STATE[key]


def _dmajor(a):
    """[d(256), n] f32 -> [128, 2, n] contiguous (d = h*128 + p)."""
    n = a.shape[1]
    return np.ascontiguousarray(a.reshape(2, P, n).transpose(1, 0, 2))


def _in_maps(x_decoder, x_encoder, Wq, bq, Wk, bk, Wv, bv, Wo, bo):
    import ml_dtypes
    f8 = ml_dtypes.float8_e4m3

    x_decoder = np.asarray(x_decoder, dtype=np.float32)
    x_encoder = np.asarray(x_encoder, dtype=np.float32)
    Wq, Wk, Wv, Wo = (np.asarray(w, dtype=np.float32) for w in (Wq, Wk, Wv, Wo))
    bq, bk, bv, bo = (np.asarray(b, dtype=np.float32) for b in (bq, bk, bv, bo))
    bo2 = (bv.astype(np.float64) @ Wo.astype(np.float64)
           + bo.astype(np.float64)).astype(np.float32)
    Wvo = (Wv.astype(np.float64) @ Wo.astype(np.float64)).astype(np.float32)

    # weight layouts [di_lo(128part), di_hi(2), do(256)] in fp8
    wq8 = np.ascontiguousarray(Wq.reshape(2, P, D).transpose(1, 0, 2)).astype(f8)
    wk8 = np.ascontiguousarray(Wk.reshape(2, P, D).transpose(1, 0, 2)).astype(f8)
    wvo8 = np.ascontiguousarray(Wvo.reshape(2, P, D).transpose(1, 0, 2)).astype(f8)
    bq2 = np.ascontiguousarray(bq.reshape(2, P).T)
    bk2 = np.ascontiguousarray(bk.reshape(2, P).T)

    maps = []
    for c in range(NCORES):
        b, h = divmod(c, 2)
        xd = x_decoder[b, h * QCHUNK:(h + 1) * QCHUNK]        # [2048, 256]
        xdT = np.ascontiguousarray(xd.T)                      # [256, 2048]
        xeT = np.ascontiguousarray(x_encoder[b].T)            # [256, 4096]
        maps.append({
            "xdT8": _dmajor(xdT).astype(f8),
            "xdT32": _dmajor(xdT + bo2[:, None]),
            "xeT8": _dmajor(xeT).astype(f8),
            "wq8": wq8, "wk8": wk8, "wvo8": wvo8,
            "bq2": bq2, "bk2": bk2,
        })
    return maps


def _assemble(results):
    out = np.empty((B, LQ, D), dtype=np.float32)
    for c in range(NCORES):
        b, h = divmod(c, 2)
        outT = results[c]["outT"].reshape(D, QCHUNK)
        out[b, h * QCHUNK:(h + 1) * QCHUNK] = outT.T
    return out


def _get_compiled(repeat=1):
    """Build a reusable jitted SPMD executable (compiles once per process)."""
    ckey = f"compiled{repeat}"
    if ckey in _STATE:
        return _STATE[ckey]
    import jax
    import numpy as jnp_np
    from jax.sharding import Mesh, PartitionSpec
    from jax.experimental.shard_map import shard_map
    from concourse import bass2jax, mybir

    nc = _get_nc(repeat)
    bass2jax.install_neuronx_cc_hook()
    partition_name = (nc.partition_id_tensor.name
                      if nc.partition_id_tensor else None)
    in_names, out_names, out_avals, zero_outs = [], [], [], []
    for alloc in nc.m.functions[0].allocations:
        if not isinstance(alloc, mybir.MemoryLocationSet):
            continue
        name = alloc.memorylocations[0].name
        if alloc.kind == "ExternalInput":
            if name != partition_name:
                in_names.append(name)
        elif alloc.kind == "ExternalOutput":
            shape = tuple(alloc.tensor_shape)
            dtype = mybir.dt.np(alloc.dtype)
            out_names.append(name)
            out_avals.append(jax.core.ShapedArray(shape, dtype))
            zero_outs.append(np.zeros((NCORES * shape[0], *shape[1:]), dtype))
    n_params = len(in_names)
    all_names = in_names + out_names
    if partition_name is not None:
        all_names.append(partition_name)

    def _body(*args):
        operands = list(args)
        if partition_name is not None:
            operands.append(bass2jax.partition_id_tensor())
        outs = bass2jax._bass_exec_p.bind(
            *operands,
            out_avals=tuple(out_avals),
            in_names=tuple(all_names),
            out_names=tuple(out_names),
            lowering_input_output_aliases=(),
            sim_require_finite=True,
            sim_require_nnan=True,
            nc=nc,
        )
        return tuple(outs)

    devices = jax.devices()[:NCORES]
    mesh = Mesh(jnp_np.asarray(devices), ("core",))
    nio = n_params + len(out_names)
    sharded = jax.jit(
        shard_map(_body, mesh=mesh,
                  in_specs=(PartitionSpec("core"),) * nio,
                  out_specs=(PartitionSpec("core"),) * len(out_names),
                  check_rep=False),
        keep_unused=True,
    )
    _STATE[ckey] = (sharded, in_names, out_names, out_avals, zero_outs, mesh)
    return _STATE[ckey]


def _concat_inputs(maps, in_names):
    return [np.concatenate([maps[c][n] for c in range(NCORES)], axis=0)
            for n in in_names]


def run_maps(maps):
    sharded, in_names, out_names, out_avals, zero_outs, mesh = _get_compiled()
    concat_in = _concat_inputs(maps, in_names)
    out_arrs = sharded(*concat_in, *zero_outs)
    results = []
    for c in range(NCORES):
        results.append({
            name: np.asarray(out_arrs[i]).reshape(NCORES, *out_avals[i].shape)[c]
            for i, name in enumerate(out_names)})
    return results


def kernel(x_decoder, x_encoder, Wq, bq, Wk, bk, Wv, bv, Wo, bo):
    maps = _in_maps(x_decoder, x_encoder, Wq, bq, Wk, bk, Wv, bv, Wo, bo)
    return _assemble(run_maps(maps))


def bench(maps, iters=30, repeat=1):
    """Time repeated executions with device-resident inputs; returns seconds/iter."""
    import time

    import jax
    from jax.sharding import NamedSharding, PartitionSpec

    sharded, in_names, out_names, out_avals, zero_outs, mesh = _get_compiled(repeat)
    sh = NamedSharding(mesh, PartitionSpec("core"))
    dev_in = [jax.device_put(a, sh) for a in _concat_inputs(maps, in_names)]
    dev_zero = [jax.device_put(z, sh) for z in zero_outs]
    jax.block_until_ready(dev_in + dev_zero)
    out = sharded(*dev_in, *dev_zero)
    jax.block_until_ready(out)
    times = []
    for _ in range(iters):
        t0 = time.perf_counter()
        out = sharded(*dev_in, *dev_zero)
        jax.block_until_ready(out)
        times.append(time.perf_counter() - t0)
    times.sort()
    return {"min": times[0], "median": times[len(times) // 2],
            "mean": sum(times) / len(times)}
